# revision 13
# baseline (speedup 1.0000x reference)
"""GAT (3-layer, 4-head) + global mean pool + FC on 8 Trainium2 NeuronCores.

Strategy (v2)
-------------
Nodes sharded contiguously across 8 cores (2500 each, padded to 2560; `batch`
is sorted so this is graph-aligned data parallelism). Per layer:
  1. Dense phase: H^T = W^T X^T on the PE; attention logits a_src/a_dst per
     node; H^T transposed into a per-node row table [h(interleaved)|a_src]
     and AllGathered so every core holds the full 20480-row table in HBM.
  2. Agg phase, per 128-dst window: ONE dma_gather fetches h+a_src rows for
     the window's edges (sorted by dst, chunked 128/chunk). Host-precomputed
     one-hot scatter matrices O / O^T (static edge structure) stream in via
     HWDGE. a_dst per edge = O^T @ a_dst_window on the PE; q = exp(lrelu(
     a_s+a_d)) on ACT; q folded into gathered rows with a single broadcast
     multiply per chunk (head-interleaved channels); PE contracts
     out[dst,c] += O^T(edges->dst) . (q*h) plus denominators. Normalize +
     ELU fused on full-width [128,512] ops with per-partition reciprocals.
  3. Next layer's X^T obtained by HWDGE dma-transpose of the row output.
Final: graph mean-pool fused into layer-3 agg windows, AllReduce, FC.
"""
import os
import sys

sys.path.insert(0, "/opt/trn_rl_repo")

import ml_dtypes
import numpy as np

import concourse.bass as bass
import concourse.tile as tile
from concourse import bacc, mybir
from concourse.bass_utils import run_bass_kernel_spmd

# problem constants (hardcoded per the harness contract)
N = 20000
E0 = 320000
IN_CH = 256
HID = 128
HEADS = 4
OUT_CH = 200
G = 64
NEG_SLOPE = 0.2
NCORES = 8
SH = N // NCORES          # 2500 nodes per core
NW = (SH + 127) // 128    # 20 windows per core
SHP = NW * 128            # 2560 padded shard
NP = NCORES * SHP         # 20480 padded global rows
P = 128
ROW1 = 640                # table row cols (bf16) for layers 1-2: 512 h + 8 as-f32 + pad
ROW3 = 256                # layer 3: 128 h + 2 as-f32 + pad

F32 = mybir.dt.float32
BF16 = mybir.dt.bfloat16
I16 = mybir.dt.int16
BF = ml_dtypes.bfloat16

AluOp = mybir.AluOpType
Act = mybir.ActivationFunctionType


# ----------------------------------------------------------------- host prep
def preprocess(edge_index, batch):
    src = edge_index[0].astype(np.int64)
    dst = edge_index[1].astype(np.int64)
    order = np.argsort(dst, kind="stable")
    src_s = src[order]
    dst_s = dst[order]

    core = dst_s // SH
    win = (dst_s % SH) // 128
    group = core * NW + win                      # 0..159, nondecreasing
    counts = np.bincount(group, minlength=NCORES * NW)
    K = int(np.ceil(counts.max() / 128))
    KW = [
        max(1, int(np.ceil(counts.reshape(NCORES, NW)[:, w].max() / 128)))
        for w in range(NW)
    ]
    SLOTS = NW * K * 128

    starts = np.zeros(NCORES * NW, np.int64)
    starts[1:] = np.cumsum(counts)[:-1]
    rank = np.arange(len(dst_s)) - starts[group]
    slot = group * (K * 128) + rank              # global slot id

    SRC = np.zeros(NCORES * SLOTS, np.int64)
    DCOL = np.full(NCORES * SLOTS, -1, np.int64)
    # remap src node id into the padded 2560-per-core row space
    SRC[slot] = (src_s // SH) * SHP + (src_s % SH)
    DCOL[slot] = dst_s - core * SH - win * 128

    def wrap16(a):
        # slot i -> [i%16, i//16], replicated to 128 partitions
        w = a.reshape(-1, 16).T.astype(np.int16)     # [16, SLOTS/16]
        return np.ascontiguousarray(np.tile(w, (8, 1)))

    dkeys = np.arange(128, dtype=np.int64)
    per_core = []
    for c in range(NCORES):
        sl = slice(c * SLOTS, (c + 1) * SLOTS)
        srcidx = wrap16(SRC[sl])                      # [128, SLOTS/16] i16
        dcol = DCOL[sl].reshape(NW, K, 128)           # [w, k, p]
        # O[p, w, k, d] = 1 iff dst col of slot (w,k,p) == d
        oh = (dcol[:, :, :, None] == dkeys).astype(np.float32)  # [w,k,p,d]
        O = np.ascontiguousarray(oh.transpose(2, 0, 1, 3)).astype(BF)
        OT = np.ascontiguousarray(oh.transpose(3, 0, 1, 2)).astype(BF)
        nodes = c * SH + np.arange(SHP)
        gid = np.where(nodes < (c + 1) * SH, batch[np.minimum(nodes, N - 1)], -1)
        gidcol = gid.reshape(NW, 128).T.astype(np.float32)  # [128, NW]
        per_core.append(dict(srcidx=srcidx, O=O, OT=OT, gidcol=gidcol))
    cnts = np.bincount(batch.astype(np.int64), minlength=G).astype(np.float32)
    invcnt = (1.0 / np.maximum(cnts, 1.0)).reshape(G, 1)
    return K, KW, per_core, invcnt


# ------------------------------------------------------------ device program
def build_program(K, KW):
    nc = bacc.Bacc("TRN2", num_devices=NCORES)
    IDXW = NW * K * 128 // 16   # idx cols per core

    # ---- inputs
    xT0 = nc.dram_tensor("xT0", [P, 2, SHP], BF16, kind="ExternalInput")
    w1 = nc.dram_tensor("w1", [P, 2, 512], BF16, kind="ExternalInput")
    w2 = nc.dram_tensor("w2", [P, 4, 512], BF16, kind="ExternalInput")
    w3 = nc.dram_tensor("w3", [P, 4, 128], BF16, kind="ExternalInput")
    a1 = nc.dram_tensor("a1", [P, 8], BF16, kind="ExternalInput")
    a2 = nc.dram_tensor("a2", [P, 8], BF16, kind="ExternalInput")
    a3 = nc.dram_tensor("a3", [P, 2], BF16, kind="ExternalInput")
    srcidx_d = nc.dram_tensor("srcidx", [P, IDXW], I16, kind="ExternalInput")
    O_d = nc.dram_tensor("Omat", [P, NW, K, 128], BF16, kind="ExternalInput")
    OT_d = nc.dram_tensor("OTmat", [P, NW, K, 128], BF16, kind="ExternalInput")
    gidcol_d = nc.dram_tensor("gidcol", [P, NW], F32, kind="ExternalInput")
    iota64_d = nc.dram_tensor("iota64", [P, G], BF16, kind="ExternalInput")
    idbf_d = nc.dram_tensor("idbf", [P, P], BF16, kind="ExternalInput")
    idf32_d = nc.dram_tensor("idf32", [P, P], F32, kind="ExternalInput")
    invcnt_d = nc.dram_tensor("invcnt", [G, 1], F32, kind="ExternalInput")
    fcw_d = nc.dram_tensor("fcw", [HID, OUT_CH], F32, kind="ExternalInput")
    out_d = nc.dram_tensor("logits", [G, OUT_CH], F32, kind="ExternalOutput")
    hdump_d = nc.dram_tensor("hdump", [NP, ROW1], BF16, kind="ExternalOutput")
    xdump_d = nc.dram_tensor("xdump", [SHP, 512], BF16, kind="ExternalOutput")

    # ---- DRAM internals / collective buffers
    hag_in = [
        nc.dram_tensor(f"hag_in{l}", [SHP, ROW1 if l < 2 else ROW3], BF16)
        for l in range(3)
    ]
    hag_out = [
        nc.dram_tensor(
            f"hag_out{l}", [NP, ROW1 if l < 2 else ROW3], BF16,
            addr_space="Shared",
        )
        for l in range(3)
    ]
    xrows = [nc.dram_tensor(f"xrows{l}", [SHP, 512], BF16) for l in (1, 2)]
    ar_in = nc.dram_tensor("ar_in", [G, HID], F32)
    ar_out = nc.dram_tensor("ar_out", [G, HID], F32, addr_space="Shared")

    RG = [list(range(NCORES))]
    NT = SHP // 512  # 5 tiles per shard in the dense phase
    dma_sem = nc.alloc_semaphore("swdge_dma")

    with tile.TileContext(nc) as tc:
        with (
            tc.tile_pool(name="const", bufs=1) as cpool,
            tc.tile_pool(name="xbuf", bufs=1) as xpool,
            tc.tile_pool(name="dense", bufs=2) as dpool,
            tc.tile_pool(name="gather", bufs=2) as gpool,
            tc.tile_pool(name="work", bufs=2) as wpool,
            tc.tile_pool(name="psA", bufs=1, space="PSUM") as psA,
        ):
            # ---- constants into SBUF
            def load_const(dram, shape, dt, name):
                t = cpool.tile(shape, dt, name=name)
                nc.sync.dma_start(t[:], dram[:])
                return t

            iota64 = load_const(iota64_d, [P, G], BF16, "iota64")
            idbf = load_const(idbf_d, [P, P], BF16, "idbf")
            idf32 = load_const(idf32_d, [P, P], F32, "idf32")
            invcnt = load_const(invcnt_d, [G, 1], F32, "invcnt")
            fcw = load_const(fcw_d, [HID, OUT_CH], F32, "fcw")
            a_sb = [
                load_const(d, [P, n], BF16, f"a{i+1}")
                for i, (d, n) in enumerate([(a1, 8), (a2, 8), (a3, 2)])
            ]
            w_sb = [
                load_const(w1, [P, 2, 512], BF16, "w1"),
                load_const(w2, [P, 4, 512], BF16, "w2"),
                load_const(w3, [P, 4, 128], BF16, "w3"),
            ]
            srcidx = load_const(srcidx_d, [P, IDXW], I16, "srcidx")
            gidcol = load_const(gidcol_d, [P, NW], F32, "gidcol")
            zero1 = cpool.tile([P, 1], F32, name="zero1")
            nc.vector.memset(zero1[:], 0.0)

            # persistent SBUF buffers
            xT = xpool.tile([P, 4, SHP], BF16, name="xT")
            nc.sync.dma_start(xT[:, 0:2, :], xT0[:])
            ht = xpool.tile([P, 4, SHP], BF16, name="ht")
            adrec = xpool.tile([P, NW, HEADS], BF16, name="adrec")

            def dense_phase(l):
                """X^T -> H^T; alpha logits; row table -> hag_in; AllGather."""
                cinb = [2, 4, 4][l]
                coutb = [4, 4, 1][l]
                nh = [4, 4, 1][l]
                rowc = ROW1 if l < 2 else ROW3
                # H^T = W^T @ X^T
                for co in range(coutb):
                    for t in range(NT):
                        pm = psA.tile([P, 512], F32, tag=f"a{t % 2}",
                                      name=f"mm{l}_{co}_{t}")
                        for k in range(cinb):
                            nc.tensor.matmul(
                                out=pm[:],
                                lhsT=w_sb[l][:, k, co * 128: co * 128 + 128],
                                rhs=xT[:, k, t * 512: (t + 1) * 512],
                                start=(k == 0),
                                stop=(k == cinb - 1),
                            )
                        nc.vector.tensor_copy(
                            ht[:, co, t * 512: (t + 1) * 512], pm[:]
                        )
                # alpha logits [2, SHP] f32 per head
                ast_h = []
                for h in range(nh):
                    ah = dpool.tile([2, SHP], BF16, tag=f"ast{h}", bufs=1,
                                    name=f"ast{l}_{h}")
                    for t in range(NT):
                        pa = psA.tile([2, 512], F32, tag=f"c{t % 2}",
                                      name=f"aps{l}_{h}_{t}")
                        nc.tensor.matmul(
                            out=pa[:],
                            lhsT=a_sb[l][:, 2 * h: 2 * h + 2],
                            rhs=ht[:, h, t * 512: (t + 1) * 512],
                            start=True,
                            stop=True,
                        )
                        nc.vector.tensor_copy(ah[:, t * 512: (t + 1) * 512], pa[:])
                    ast_h.append(ah)
                # per-window: transpose H^T into interleaved rows + a-records
                for w in range(NW):
                    ws = slice(w * 128, (w + 1) * 128)
                    rows = dpool.tile([P, rowc], BF16, tag="rows",
                                      name=f"rows{l}_{w}")
                    rview = (
                        rows[:, 0:512].rearrange("p (c h) -> p c h", h=4)
                        if nh == 4 else rows[:, 0:128]
                    )
                    for co in range(coutb):
                        pt = psA.tile([P, P], BF16, tag=f"b{co % 2}",
                                      name=f"htp{l}_{w}_{co}")
                        nc.tensor.matmul(
                            out=pt[:], lhsT=ht[:, co, ws], rhs=idbf[:],
                            start=True, stop=True, is_transpose=True,
                        )
                        if nh == 4:
                            nc.vector.tensor_copy(rview[:, :, co], pt[:])
                        else:
                            nc.vector.tensor_copy(rview[:, :], pt[:])
                    # a_src / a_dst records: transpose [2,128] -> [128,2] bf16
                    for h in range(nh):
                        pr = psA.tile([P, 2], BF16, tag=f"c{h % 2}",
                                      name=f"arec{l}_{w}_{h}")
                        nc.tensor.matmul(
                            out=pr[:], lhsT=ast_h[h][:, ws], rhs=idbf[:2, :2],
                            start=True, stop=True, is_transpose=True,
                        )
                        nc.vector.tensor_copy(
                            rows[:, 512 + h: 513 + h] if l < 2
                            else rows[:, 128:129],
                            pr[:, 0:1],
                        )
                        nc.vector.tensor_copy(adrec[:, w, h: h + 1], pr[:, 1:2])
                    nc.sync.dma_start(hag_in[l][ws, :], rows[:])
                nc.gpsimd.collective_compute(
                    "AllGather", AluOp.bypass, replica_groups=RG,
                    ins=[hag_in[l][:]], outs=[hag_out[l][:]],
                )

            def agg_phase(l, pool_ps=None):
                """Gather + attention + scatter; rows out (elu'd)."""
                nh = [4, 4, 1][l]
                C = [512, 512, 128][l]
                rowc = ROW1 if l < 2 else ROW3
                for w in range(NW):
                    Kw = KW[w]
                    NI = Kw * 128
                    isl = slice(w * K * 8, w * K * 8 + Kw * 8)
                    hg = gpool.tile([P, Kw, rowc], BF16, tag="hg", name=f"hg{l}_{w}")
                    nc.gpsimd.dma_gather(
                        hg[:], hag_out[l][:], srcidx[:, isl], NI, NI, rowc,
                        single_packet=False,
                    )
                    Ow = gpool.tile([P, Kw, 128], BF16, tag="Ow", name=f"O{l}_{w}")
                    nc.sync.dma_start(Ow[:], O_d[:, w, 0:Kw, :])
                    OTw = gpool.tile([P, Kw, 128], BF16, tag="OTw", name=f"OT{l}_{w}")
                    nc.sync.dma_start(OTw[:], OT_d[:, w, 0:Kw, :])
                    # a_dst per edge via O^T @ ad_window  -> [128, K, nh] psum
                    adps = psA.tile([P, Kw * nh], F32, tag=f"b{w % 2}",
                                    name=f"adps{l}_{w}")
                    for k in range(Kw):
                        nc.tensor.matmul(
                            out=adps[:, k * nh: (k + 1) * nh],
                            lhsT=OTw[:, k, :], rhs=adrec[:, w, 0:nh],
                            start=True, stop=True,
                        )
                    # q = exp(lrelu(as + ad)) -> bf16 [128, K, nh]
                    asv = (
                        hg[:, :, 512:516] if l < 2 else hg[:, :, 128:129]
                    )  # [128, K, nh] bf16
                    tq = wpool.tile([P, Kw, nh], F32, tag="tq", name=f"tq{l}_{w}")
                    nc.vector.tensor_tensor(
                        out=tq[:], in0=asv,
                        in1=adps[:].rearrange("p (k h) -> p k h", h=nh),
                        op=AluOp.add,
                    )
                    ql = wpool.tile([P, Kw, nh], F32, tag="ql", name=f"ql{l}_{w}")
                    nc.vector.scalar_tensor_tensor(
                        out=ql[:], in0=tq[:], scalar=NEG_SLOPE, in1=tq[:],
                        op0=AluOp.mult, op1=AluOp.max,
                    )
                    qf = wpool.tile([P, Kw, nh], BF16, tag="qf", name=f"qf{l}_{w}")
                    nc.scalar.activation(qf[:], ql[:], Act.Exp)
                    if nh == 1:
                        qf32 = wpool.tile([P, Kw, 1], F32, tag="qf32",
                                          name=f"qf32{l}_{w}")
                        nc.scalar.activation(qf32[:], ql[:], Act.Exp)
                    # hgs = hg * q (broadcast over channels), per chunk
                    hgs = wpool.tile([P, Kw, C], BF16, tag="hgs", bufs=1, name=f"hgs{l}_{w}")
                    pagg = psA.tile([P, C], F32, tag=f"a{w % 2}", name=f"pagg{l}_{w}")
                    den = psA.tile([P, nh], F32, tag=f"c{w % 2}", name=f"den{l}_{w}")
                    for k in range(Kw):
                        if nh == 4:
                            nc.vector.tensor_tensor(
                                out=hgs[:, k, :].rearrange("p (c h) -> p c h", h=4),
                                in0=hg[:, k, 0:512].rearrange("p (c h) -> p c h", h=4),
                                in1=qf[:, k, :].unsqueeze(1).broadcast_to(
                                    [P, 128, 4]
                                ),
                                op=AluOp.mult,
                            )
                        else:
                            nc.vector.tensor_tensor(
                                out=hgs[:, k, :], in0=hg[:, k, 0:128],
                                in1=qf32[:, k, 0:1].broadcast_to([P, 128]),
                                op=AluOp.mult,
                            )
                        nc.tensor.matmul(
                            out=pagg[:], lhsT=Ow[:, k, :], rhs=hgs[:, k, :],
                            start=(k == 0), stop=(k == Kw - 1),
                        )
                        nc.tensor.matmul(
                            out=den[:], lhsT=Ow[:, k, :], rhs=qf[:, k, :],
                            start=(k == 0), stop=(k == Kw - 1),
                        )
                    # self-loop term from local rows (no gather needed)
                    hw_ = wpool.tile([P, rowc], BF16, tag="hw", name=f"hw{l}_{w}")
                    nc.sync.dma_start(hw_[:], hag_in[l][w * 128:(w + 1) * 128, :])
                    asw = hw_[:, 512:516] if l < 2 else hw_[:, 128:129]
                    tqs = wpool.tile([P, nh], F32, tag="tqs", name=f"tqs{l}_{w}")
                    nc.vector.tensor_tensor(
                        out=tqs[:], in0=asw, in1=adrec[:, w, 0:nh], op=AluOp.add,
                    )
                    qls = wpool.tile([P, nh], F32, tag="qls", name=f"qls{l}_{w}")
                    nc.vector.scalar_tensor_tensor(
                        out=qls[:], in0=tqs[:], scalar=NEG_SLOPE, in1=tqs[:],
                        op0=AluOp.mult, op1=AluOp.max,
                    )
                    qs = wpool.tile([P, nh], F32, tag="qs", name=f"qs{l}_{w}")
                    nc.scalar.activation(qs[:], qls[:], Act.Exp)
                    smsg = wpool.tile([P, C], F32, tag="smsg", bufs=1, name=f"sm{l}_{w}")
                    if nh == 4:
                        nc.vector.tensor_tensor(
                            out=smsg[:].rearrange("p (c h) -> p c h", h=4),
                            in0=hw_[:, 0:512].rearrange("p (c h) -> p c h", h=4),
                            in1=qs[:].unsqueeze(1).broadcast_to([P, 128, 4]),
                            op=AluOp.mult,
                        )
                    else:
                        nc.vector.tensor_tensor(
                            out=smsg[:], in0=hw_[:, 0:128],
                            in1=qs[:, 0:1].broadcast_to([P, 128]),
                            op=AluOp.mult,
                        )
                    # normalize + elu -> rows (bf16)
                    rec = wpool.tile([P, nh], F32, tag="rec", name=f"rec{l}_{w}")
                    nc.vector.scalar_tensor_tensor(
                        out=rec[:], in0=den[:], scalar=1e-16, in1=qs[:],
                        op0=AluOp.add, op1=AluOp.add,
                    )
                    nc.vector.reciprocal(rec[:], rec[:])
                    padd = wpool.tile([P, C], F32, tag="padd", bufs=1, name=f"pd{l}_{w}")
                    nc.vector.scalar_tensor_tensor(
                        out=padd[:], in0=pagg[:], scalar=0.0, in1=smsg[:],
                        op0=AluOp.add, op1=AluOp.add,
                    )
                    tmul = wpool.tile([P, C], F32, tag="tmul", bufs=1, name=f"tm{l}_{w}")
                    if nh == 4:
                        nc.vector.tensor_tensor(
                            out=tmul[:].rearrange("p (c h) -> p c h", h=4),
                            in0=padd[:].rearrange("p (c h) -> p c h", h=4),
                            in1=rec[:].unsqueeze(1).broadcast_to([P, 128, 4]),
                            op=AluOp.mult,
                        )
                    else:
                        nc.vector.tensor_tensor(
                            out=tmul[:], in0=padd[:],
                            in1=rec[:, 0:1].broadcast_to([P, 128]),
                            op=AluOp.mult,
                        )
                    tmin = wpool.tile([P, C], F32, tag="tmin", bufs=1, name=f"tn{l}_{w}")
                    nc.vector.scalar_tensor_tensor(
                        out=tmin[:], in0=tmul[:], scalar=0.0,
                        in1=zero1[:, 0:1].broadcast_to([P, C]),
                        op0=AluOp.add, op1=AluOp.min,
                    )
                    em = wpool.tile([P, C], F32, tag="em", bufs=1, name=f"em{l}_{w}")
                    nc.scalar.activation(em[:], tmin[:], Act.Exp)
                    relu = wpool.tile([P, C], F32, tag="relu", bufs=1, name=f"rl{l}_{w}")
                    nc.vector.scalar_tensor_tensor(
                        out=relu[:], in0=tmul[:], scalar=0.0,
                        in1=zero1[:, 0:1].broadcast_to([P, C]),
                        op0=AluOp.add, op1=AluOp.max,
                    )
                    orow = wpool.tile([P, C], BF16, tag="orow", name=f"or{l}_{w}")
                    nc.vector.scalar_tensor_tensor(
                        out=orow[:], in0=em[:], scalar=-1.0, in1=relu[:],
                        op0=AluOp.add, op1=AluOp.add,
                    )
                    if l < 2:
                        nc.sync.dma_start(xrows[l][w * 128:(w + 1) * 128, :], orow[:])
                    else:
                        # fuse graph pooling: pool_ps += gsel^T @ rows
                        gw = wpool.tile([P, G], BF16, tag="gw", name=f"gw_{w}")
                        nc.vector.tensor_tensor(
                            out=gw[:], in0=iota64[:],
                            in1=gidcol[:, w: w + 1].broadcast_to([P, G]),
                            op=AluOp.is_equal,
                        )
                        nc.tensor.matmul(
                            out=pool_ps[:], lhsT=gw[:], rhs=orow[:],
                            start=(w == 0), stop=(w == NW - 1),
                        )

            def load_xT(l):
                """X^T for layer l in {1,2} via HWDGE dma-transpose of rows."""
                for b in range(4):
                    nc.sync.dma_start_transpose(
                        xT[:, b, :], xrows[l - 1][:, b * 128:(b + 1) * 128]
                    )

            def pool_fc(pool_ps):
                psums = wpool.tile([G, HID], F32, tag="psums", name="psums")
                nc.vector.tensor_copy(psums[:], pool_ps[:])
                nc.sync.dma_start(ar_in[:], psums[:])
                nc.gpsimd.collective_compute(
                    "AllReduce", AluOp.add, replica_groups=RG,
                    ins=[ar_in[:]], outs=[ar_out[:]],
                )
                sums = wpool.tile([G, HID], F32, tag="sums", name="sums")
                nc.sync.dma_start(sums[:], ar_out[:])
                pooled = wpool.tile([G, HID], F32, tag="pooled", name="pooled")
                nc.vector.tensor_scalar(
                    out=pooled[:], in0=sums[:], scalar1=invcnt[:, 0:1],
                    scalar2=None, op0=AluOp.mult,
                )
                ptp = psA.tile([HID, G], F32, tag="c0", name="poolT")
                nc.tensor.matmul(
                    out=ptp[:], lhsT=pooled[:], rhs=idf32[:G, :G],
                    start=True, stop=True, is_transpose=True,
                )
                poolT = wpool.tile([HID, G], F32, tag="poolT", name="poolTs")
                nc.vector.tensor_copy(poolT[:], ptp[:])
                pfc = psA.tile([G, OUT_CH], F32, tag="b0", name="fcps")
                nc.tensor.matmul(
                    out=pfc[:], lhsT=poolT[:], rhs=fcw[:], start=True, stop=True
                )
                logits = wpool.tile([G, OUT_CH], F32, tag="logits", name="logits")
                nc.vector.tensor_copy(logits[:], pfc[:])
                nc.sync.dma_start(out_d[:], logits[:])

            dense_phase(0)
            agg_phase(0)
            load_xT(1)
            dense_phase(1)
            agg_phase(1)
            load_xT(2)
            dense_phase(2)
            pool_ps = psA.tile([G, HID], F32, tag="d0", name="poolps")
            agg_phase(2, pool_ps)
            pool_fc(pool_ps)

            if os.environ.get("DUMP_H"):
                li = int(os.environ["DUMP_H"])
                cw = ROW1 if li < 2 else ROW3
                hstg = wpool.tile([P, cw], BF16, tag="hdmp", bufs=2, name="hdmp")
                for b in range(NP // P):
                    lo, hi = b * P, (b + 1) * P
                    nc.sync.dma_start(hstg[:], hag_out[li][lo:hi, :])
                    nc.sync.dma_start(hdump_d[lo:hi, 0:cw], hstg[:])
            if os.environ.get("DUMP_X"):
                xi = int(os.environ["DUMP_X"])  # 1 or 2: xrows after agg xi-1
                xstg = wpool.tile([P, 512], BF16, tag="xdmp", bufs=2, name="xdmp")
                for b in range(SHP // P):
                    lo, hi = b * P, (b + 1) * P
                    nc.sync.dma_start(xstg[:], xrows[xi - 1][lo:hi, :])
                    nc.sync.dma_start(xdump_d[lo:hi, :], xstg[:])

    nc.compile()
    return nc


_prog_cache = {}


def _interleave_perm():
    # perm[j] = flat channel index stored at interleaved col j
    j = np.arange(512)
    c, h = j // 4, j % 4
    return h * 128 + c


def kernel(x, edge_index, batch, W1, a_src1, a_dst1, b1,
           W2, a_src2, a_dst2, b2, W3, a_src3, a_dst3, b3, fc_w, fc_b,
           _want_results=False, _trace=False):
    x = np.asarray(x)
    edge_index = np.asarray(edge_index)
    batch = np.asarray(batch)
    for b in (b1, b2, b3, fc_b):
        assert not np.any(np.asarray(b)), "nonzero biases not supported"

    K, KW, per_core, invcnt = preprocess(edge_index, batch)
    ck = (K, tuple(KW))
    if ck not in _prog_cache:
        _prog_cache[ck] = build_program(K, KW)
    nc = _prog_cache[ck]

    iota64 = np.ascontiguousarray(
        np.broadcast_to(np.arange(G, dtype=np.float32), (P, G)).astype(BF)
    )
    idbf = np.eye(P, dtype=np.float32).astype(BF)
    idf32 = np.eye(P, dtype=np.float32)
    perm = _interleave_perm()

    def wmat(W, cinb, cout, perm_in=None):
        Wf = np.asarray(W, np.float32)
        if perm_in is not None:
            Wf = Wf[perm_in]
        return np.ascontiguousarray(
            Wf.reshape(cinb, 128, cout).transpose(1, 0, 2)
        ).astype(BF)

    w1m = wmat(W1, 2, 512)
    w2m = wmat(W2, 4, 512, perm)
    w3m = wmat(W3, 4, 128, perm)

    def avec(asrc, adst):
        nh = asrc.shape[0]
        out = np.empty((128, 2 * nh), np.float32)
        out[:, 0::2] = np.asarray(asrc, np.float32).T
        out[:, 1::2] = np.asarray(adst, np.float32).T
        return np.ascontiguousarray(out).astype(BF)

    a1m = avec(a_src1, a_dst1)
    a2m = avec(a_src2, a_dst2)
    a3m = avec(a_src3, a_dst3)
    fcw = np.ascontiguousarray(np.asarray(fc_w, np.float32))

    xf = np.asarray(x, np.float32)
    in_maps = []
    for c in range(NCORES):
        xs = np.zeros((IN_CH, SHP), np.float32)
        xs[:, :SH] = xf[c * SH: (c + 1) * SH].T
        pc = per_core[c]
        in_maps.append(
            dict(
                xT0=np.ascontiguousarray(
                    xs.reshape(2, 128, SHP).transpose(1, 0, 2)
                ).astype(BF),
                w1=w1m, w2=w2m, w3=w3m, a1=a1m, a2=a2m, a3=a3m,
                srcidx=pc["srcidx"], Omat=pc["O"], OTmat=pc["OT"],
                gidcol=pc["gidcol"],
                iota64=iota64, idbf=idbf, idf32=idf32, invcnt=invcnt, fcw=fcw,
            )
        )
    res = run_bass_kernel_spmd(
        nc, in_maps, list(range(NCORES)), trace=_trace
    )
    out = res.results[0]["logits"].astype(np.float32)
    if _want_results:
        return out, res
    return out


# revision 15
# speedup vs baseline: 1.0183x; 1.0183x over previous
"""GAT (3-layer, 4-head) + global mean pool + FC on 8 Trainium2 NeuronCores.

Strategy (v2)
-------------
Nodes sharded contiguously across 8 cores (2500 each, padded to 2560; `batch`
is sorted so this is graph-aligned data parallelism). Per layer:
  1. Dense phase: H^T = W^T X^T on the PE; attention logits a_src/a_dst per
     node; H^T transposed into a per-node row table [h(interleaved)|a_src]
     and AllGathered so every core holds the full 20480-row table in HBM.
  2. Agg phase, per 128-dst window: ONE dma_gather fetches h+a_src rows for
     the window's edges (sorted by dst, chunked 128/chunk). Host-precomputed
     one-hot scatter matrices O / O^T (static edge structure) stream in via
     HWDGE. a_dst per edge = O^T @ a_dst_window on the PE; q = exp(lrelu(
     a_s+a_d)) on ACT; q folded into gathered rows with a single broadcast
     multiply per chunk (head-interleaved channels); PE contracts
     out[dst,c] += O^T(edges->dst) . (q*h) plus denominators. Normalize +
     ELU fused on full-width [128,512] ops with per-partition reciprocals.
  3. Next layer's X^T obtained by HWDGE dma-transpose of the row output.
Final: graph mean-pool fused into layer-3 agg windows, AllReduce, FC.
"""
import os
import sys

sys.path.insert(0, "/opt/trn_rl_repo")

import ml_dtypes
import numpy as np

import concourse.bass as bass
import concourse.tile as tile
from concourse import bacc, mybir
from concourse.bass_utils import run_bass_kernel_spmd

# problem constants (hardcoded per the harness contract)
N = 20000
E0 = 320000
IN_CH = 256
HID = 128
HEADS = 4
OUT_CH = 200
G = 64
NEG_SLOPE = 0.2
NCORES = 8
SH = N // NCORES          # 2500 nodes per core
NW = (SH + 127) // 128    # 20 windows per core
SHP = NW * 128            # 2560 padded shard
NP = NCORES * SHP         # 20480 padded global rows
P = 128
ROW1 = 640                # table row cols (bf16) for layers 1-2: 512 h + 8 as-f32 + pad
ROW3 = 256                # layer 3: 128 h + 2 as-f32 + pad

F32 = mybir.dt.float32
BF16 = mybir.dt.bfloat16
I16 = mybir.dt.int16
BF = ml_dtypes.bfloat16

AluOp = mybir.AluOpType
Act = mybir.ActivationFunctionType


# ----------------------------------------------------------------- host prep
def preprocess(edge_index, batch):
    src = np.concatenate([edge_index[0].astype(np.int64), np.arange(N)])
    dst = np.concatenate([edge_index[1].astype(np.int64), np.arange(N)])
    order = np.argsort(dst, kind="stable")
    src_s = src[order]
    dst_s = dst[order]

    core = dst_s // SH
    win = (dst_s % SH) // 128
    group = core * NW + win                      # 0..159, nondecreasing
    counts = np.bincount(group, minlength=NCORES * NW)
    K = int(np.ceil(counts.max() / 128))
    KW = [
        max(1, int(np.ceil(counts.reshape(NCORES, NW)[:, w].max() / 128)))
        for w in range(NW)
    ]
    SLOTS = NW * K * 128

    starts = np.zeros(NCORES * NW, np.int64)
    starts[1:] = np.cumsum(counts)[:-1]
    rank = np.arange(len(dst_s)) - starts[group]
    slot = group * (K * 128) + rank              # global slot id

    SRC = np.zeros(NCORES * SLOTS, np.int64)
    DCOL = np.full(NCORES * SLOTS, -1, np.int64)
    # remap src node id into the padded 2560-per-core row space
    SRC[slot] = (src_s // SH) * SHP + (src_s % SH)
    DCOL[slot] = dst_s - core * SH - win * 128

    def wrap16(a):
        # slot i -> [i%16, i//16], replicated to 128 partitions
        w = a.reshape(-1, 16).T.astype(np.int16)     # [16, SLOTS/16]
        return np.ascontiguousarray(np.tile(w, (8, 1)))

    dkeys = np.arange(128, dtype=np.int64)
    per_core = []
    for c in range(NCORES):
        sl = slice(c * SLOTS, (c + 1) * SLOTS)
        srcidx = wrap16(SRC[sl])                      # [128, SLOTS/16] i16
        dcol = DCOL[sl].reshape(NW, K, 128)           # [w, k, p]
        # O[p, w, k, d] = 1 iff dst col of slot (w,k,p) == d
        oh = (dcol[:, :, :, None] == dkeys).astype(np.float32)  # [w,k,p,d]
        O = np.ascontiguousarray(oh.transpose(2, 0, 1, 3)).astype(BF)
        OT = np.ascontiguousarray(oh.transpose(3, 0, 1, 2)).astype(BF)
        nodes = c * SH + np.arange(SHP)
        gid = np.where(nodes < (c + 1) * SH, batch[np.minimum(nodes, N - 1)], -1)
        gidcol = gid.reshape(NW, 128).T.astype(np.float32)  # [128, NW]
        per_core.append(dict(srcidx=srcidx, O=O, OT=OT, gidcol=gidcol))
    cnts = np.bincount(batch.astype(np.int64), minlength=G).astype(np.float32)
    invcnt = (1.0 / np.maximum(cnts, 1.0)).reshape(G, 1)
    return K, KW, per_core, invcnt


# ------------------------------------------------------------ device program
def build_program(K, KW):
    nc = bacc.Bacc("TRN2", num_devices=NCORES)
    IDXW = NW * K * 128 // 16   # idx cols per core

    # ---- inputs
    xT0 = nc.dram_tensor("xT0", [P, 2, SHP], BF16, kind="ExternalInput")
    w1 = nc.dram_tensor("w1", [P, 2, 512], BF16, kind="ExternalInput")
    w2 = nc.dram_tensor("w2", [P, 4, 512], BF16, kind="ExternalInput")
    w3 = nc.dram_tensor("w3", [P, 4, 128], BF16, kind="ExternalInput")
    a1 = nc.dram_tensor("a1", [P, 8], BF16, kind="ExternalInput")
    a2 = nc.dram_tensor("a2", [P, 8], BF16, kind="ExternalInput")
    a3 = nc.dram_tensor("a3", [P, 2], BF16, kind="ExternalInput")
    srcidx_d = nc.dram_tensor("srcidx", [P, IDXW], I16, kind="ExternalInput")
    O_d = nc.dram_tensor("Omat", [P, NW, K, 128], BF16, kind="ExternalInput")
    OT_d = nc.dram_tensor("OTmat", [P, NW, K, 128], BF16, kind="ExternalInput")
    gidcol_d = nc.dram_tensor("gidcol", [P, NW], F32, kind="ExternalInput")
    iota64_d = nc.dram_tensor("iota64", [P, G], BF16, kind="ExternalInput")
    idbf_d = nc.dram_tensor("idbf", [P, P], BF16, kind="ExternalInput")
    idf32_d = nc.dram_tensor("idf32", [P, P], F32, kind="ExternalInput")
    invcnt_d = nc.dram_tensor("invcnt", [G, 1], F32, kind="ExternalInput")
    fcw_d = nc.dram_tensor("fcw", [HID, OUT_CH], F32, kind="ExternalInput")
    out_d = nc.dram_tensor("logits", [G, OUT_CH], F32, kind="ExternalOutput")
    hdump_d = nc.dram_tensor("hdump", [NP, ROW1], BF16, kind="ExternalOutput")
    xdump_d = nc.dram_tensor("xdump", [SHP, 512], BF16, kind="ExternalOutput")

    # ---- DRAM internals / collective buffers
    hag_in = [
        nc.dram_tensor(f"hag_in{l}", [SHP, ROW1 if l < 2 else ROW3], BF16)
        for l in range(3)
    ]
    hag_out = [
        nc.dram_tensor(
            f"hag_out{l}", [NP, ROW1 if l < 2 else ROW3], BF16,
            addr_space="Shared",
        )
        for l in range(3)
    ]
    xrows = [nc.dram_tensor(f"xrows{l}", [SHP, 512], BF16) for l in (1, 2)]
    ar_in = nc.dram_tensor("ar_in", [G, HID], F32)
    ar_out = nc.dram_tensor("ar_out", [G, HID], F32, addr_space="Shared")

    RG = [list(range(NCORES))]
    NT = SHP // 512  # 5 tiles per shard in the dense phase
    dma_sem = nc.alloc_semaphore("swdge_dma")

    with tile.TileContext(nc) as tc:
        with (
            tc.tile_pool(name="const", bufs=1) as cpool,
            tc.tile_pool(name="xbuf", bufs=1) as xpool,
            tc.tile_pool(name="dense", bufs=2) as dpool,
            tc.tile_pool(name="gather", bufs=2) as gpool,
            tc.tile_pool(name="work", bufs=2) as wpool,
            tc.tile_pool(name="psA", bufs=1, space="PSUM") as psA,
        ):
            # ---- constants into SBUF
            def load_const(dram, shape, dt, name):
                t = cpool.tile(shape, dt, name=name)
                nc.sync.dma_start(t[:], dram[:])
                return t

            iota64 = load_const(iota64_d, [P, G], BF16, "iota64")
            idbf = load_const(idbf_d, [P, P], BF16, "idbf")
            idf32 = load_const(idf32_d, [P, P], F32, "idf32")
            invcnt = load_const(invcnt_d, [G, 1], F32, "invcnt")
            fcw = load_const(fcw_d, [HID, OUT_CH], F32, "fcw")
            a_sb = [
                load_const(d, [P, n], BF16, f"a{i+1}")
                for i, (d, n) in enumerate([(a1, 8), (a2, 8), (a3, 2)])
            ]
            w_sb = [
                load_const(w1, [P, 2, 512], BF16, "w1"),
                load_const(w2, [P, 4, 512], BF16, "w2"),
                load_const(w3, [P, 4, 128], BF16, "w3"),
            ]
            srcidx = load_const(srcidx_d, [P, IDXW], I16, "srcidx")
            gidcol = load_const(gidcol_d, [P, NW], F32, "gidcol")
            zero1 = cpool.tile([P, 1], F32, name="zero1")
            nc.vector.memset(zero1[:], 0.0)

            # persistent SBUF buffers
            xT = xpool.tile([P, 4, SHP], BF16, name="xT")
            nc.sync.dma_start(xT[:, 0:2, :], xT0[:])
            ht = xpool.tile([P, 4, SHP], BF16, name="ht")
            adrec = xpool.tile([P, NW, HEADS], BF16, name="adrec")

            def dense_phase(l):
                """X^T -> H^T; alpha logits; row table -> hag_in; AllGather."""
                cinb = [2, 4, 4][l]
                coutb = [4, 4, 1][l]
                nh = [4, 4, 1][l]
                rowc = ROW1 if l < 2 else ROW3
                # H^T = W^T @ X^T
                for co in range(coutb):
                    for t in range(NT):
                        pm = psA.tile([P, 512], F32, tag=f"a{t % 2}",
                                      name=f"mm{l}_{co}_{t}")
                        for k in range(cinb):
                            nc.tensor.matmul(
                                out=pm[:],
                                lhsT=w_sb[l][:, k, co * 128: co * 128 + 128],
                                rhs=xT[:, k, t * 512: (t + 1) * 512],
                                start=(k == 0),
                                stop=(k == cinb - 1),
                            )
                        nc.vector.tensor_copy(
                            ht[:, co, t * 512: (t + 1) * 512], pm[:]
                        )
                # alpha logits [2, SHP] f32 per head
                ast_h = []
                for h in range(nh):
                    ah = dpool.tile([2, SHP], BF16, tag=f"ast{h}", bufs=1,
                                    name=f"ast{l}_{h}")
                    for t in range(NT):
                        pa = psA.tile([2, 512], F32, tag=f"c{t % 2}",
                                      name=f"aps{l}_{h}_{t}")
                        nc.tensor.matmul(
                            out=pa[:],
                            lhsT=a_sb[l][:, 2 * h: 2 * h + 2],
                            rhs=ht[:, h, t * 512: (t + 1) * 512],
                            start=True,
                            stop=True,
                        )
                        nc.vector.tensor_copy(ah[:, t * 512: (t + 1) * 512], pa[:])
                    ast_h.append(ah)
                # per-window: transpose H^T into interleaved rows + a-records
                for w in range(NW):
                    ws = slice(w * 128, (w + 1) * 128)
                    rows = dpool.tile([P, rowc], BF16, tag="rows",
                                      name=f"rows{l}_{w}")
                    rview = (
                        rows[:, 0:512].rearrange("p (c h) -> p c h", h=4)
                        if nh == 4 else rows[:, 0:128]
                    )
                    for co in range(coutb):
                        pt = psA.tile([P, P], BF16, tag=f"b{co % 2}",
                                      name=f"htp{l}_{w}_{co}")
                        nc.tensor.matmul(
                            out=pt[:], lhsT=ht[:, co, ws], rhs=idbf[:],
                            start=True, stop=True, is_transpose=True,
                        )
                        if nh == 4:
                            nc.vector.tensor_copy(rview[:, :, co], pt[:])
                        else:
                            nc.vector.tensor_copy(rview[:, :], pt[:])
                    # a_src / a_dst records: transpose [2,128] -> [128,2] bf16
                    for h in range(nh):
                        pr = psA.tile([P, 2], BF16, tag=f"c{h % 2}",
                                      name=f"arec{l}_{w}_{h}")
                        nc.tensor.matmul(
                            out=pr[:], lhsT=ast_h[h][:, ws], rhs=idbf[:2, :2],
                            start=True, stop=True, is_transpose=True,
                        )
                        nc.vector.tensor_copy(
                            rows[:, 512 + h: 513 + h] if l < 2
                            else rows[:, 128:129],
                            pr[:, 0:1],
                        )
                        nc.vector.tensor_copy(adrec[:, w, h: h + 1], pr[:, 1:2])
                    nc.sync.dma_start(hag_in[l][ws, :], rows[:])
                nc.gpsimd.collective_compute(
                    "AllGather", AluOp.bypass, replica_groups=RG,
                    ins=[hag_in[l][:]], outs=[hag_out[l][:]],
                )

            def agg_phase(l, pool_ps=None):
                """Gather + attention + scatter; rows out (elu'd)."""
                nh = [4, 4, 1][l]
                C = [512, 512, 128][l]
                rowc = ROW1 if l < 2 else ROW3
                for w in range(NW):
                    Kw = KW[w]
                    NI = Kw * 128
                    isl = slice(w * K * 8, w * K * 8 + Kw * 8)
                    hg = gpool.tile([P, Kw, rowc], BF16, tag="hg", name=f"hg{l}_{w}")
                    nc.gpsimd.dma_gather(
                        hg[:], hag_out[l][:], srcidx[:, isl], NI, NI, rowc,
                        single_packet=False,
                    )
                    Ow = gpool.tile([P, Kw, 128], BF16, tag="Ow", name=f"O{l}_{w}")
                    nc.sync.dma_start(Ow[:], O_d[:, w, 0:Kw, :])
                    OTw = gpool.tile([P, Kw, 128], BF16, tag="OTw", name=f"OT{l}_{w}")
                    nc.sync.dma_start(OTw[:], OT_d[:, w, 0:Kw, :])
                    # a_dst per edge via O^T @ ad_window  -> [128, K, nh] psum
                    adps = psA.tile([P, Kw * nh], F32, tag=f"b{w % 2}",
                                    name=f"adps{l}_{w}")
                    for k in range(Kw):
                        nc.tensor.matmul(
                            out=adps[:, k * nh: (k + 1) * nh],
                            lhsT=OTw[:, k, :], rhs=adrec[:, w, 0:nh],
                            start=True, stop=True,
                        )
                    # q = exp(lrelu(as + ad)) -> bf16 [128, K, nh]
                    asv = (
                        hg[:, :, 512:516] if l < 2 else hg[:, :, 128:129]
                    )  # [128, K, nh] bf16
                    tq = wpool.tile([P, Kw, nh], F32, tag="tq", name=f"tq{l}_{w}")
                    nc.vector.tensor_tensor(
                        out=tq[:], in0=asv,
                        in1=adps[:].rearrange("p (k h) -> p k h", h=nh),
                        op=AluOp.add,
                    )
                    ql = wpool.tile([P, Kw, nh], F32, tag="ql", name=f"ql{l}_{w}")
                    nc.vector.scalar_tensor_tensor(
                        out=ql[:], in0=tq[:], scalar=NEG_SLOPE, in1=tq[:],
                        op0=AluOp.mult, op1=AluOp.max,
                    )
                    qf = wpool.tile([P, Kw, nh], BF16, tag="qf", name=f"qf{l}_{w}")
                    nc.scalar.activation(qf[:], ql[:], Act.Exp)
                    if nh == 1:
                        qf32 = wpool.tile([P, Kw, 1], F32, tag="qf32",
                                          name=f"qf32{l}_{w}")
                        nc.scalar.activation(qf32[:], ql[:], Act.Exp)
                    # hgs = hg * q (broadcast over channels), per chunk
                    hgs = wpool.tile([P, Kw, C], BF16, tag="hgs", bufs=1, name=f"hgs{l}_{w}")
                    pagg = psA.tile([P, C], F32, tag=f"a{w % 2}", name=f"pagg{l}_{w}")
                    den = psA.tile([P, nh], F32, tag=f"c{w % 2}", name=f"den{l}_{w}")
                    for k in range(Kw):
                        if nh == 4:
                            nc.vector.tensor_tensor(
                                out=hgs[:, k, :].rearrange("p (c h) -> p c h", h=4),
                                in0=hg[:, k, 0:512].rearrange("p (c h) -> p c h", h=4),
                                in1=qf[:, k, :].unsqueeze(1).broadcast_to(
                                    [P, 128, 4]
                                ),
                                op=AluOp.mult,
                            )
                        else:
                            nc.vector.tensor_tensor(
                                out=hgs[:, k, :], in0=hg[:, k, 0:128],
                                in1=qf32[:, k, 0:1].broadcast_to([P, 128]),
                                op=AluOp.mult,
                            )
                        nc.tensor.matmul(
                            out=pagg[:], lhsT=Ow[:, k, :], rhs=hgs[:, k, :],
                            start=(k == 0), stop=(k == Kw - 1),
                        )
                        nc.tensor.matmul(
                            out=den[:], lhsT=Ow[:, k, :], rhs=qf[:, k, :],
                            start=(k == 0), stop=(k == Kw - 1),
                        )
                    # normalize + elu -> rows (bf16)
                    rec = wpool.tile([P, nh], F32, tag="rec", name=f"rec{l}_{w}")
                    nc.vector.scalar_tensor_tensor(
                        out=rec[:], in0=den[:], scalar=1e-16, in1=zero1[:, 0:1].broadcast_to([P, nh]),
                        op0=AluOp.add, op1=AluOp.add,
                    )
                    nc.vector.reciprocal(rec[:], rec[:])
                    tmul = wpool.tile([P, C], F32, tag="tmul", bufs=1, name=f"tm{l}_{w}")
                    if nh == 4:
                        nc.vector.tensor_tensor(
                            out=tmul[:].rearrange("p (c h) -> p c h", h=4),
                            in0=pagg[:].rearrange("p (c h) -> p c h", h=4),
                            in1=rec[:].unsqueeze(1).broadcast_to([P, 128, 4]),
                            op=AluOp.mult,
                        )
                    else:
                        nc.vector.tensor_tensor(
                            out=tmul[:], in0=pagg[:],
                            in1=rec[:, 0:1].broadcast_to([P, 128]),
                            op=AluOp.mult,
                        )
                    tmin = wpool.tile([P, C], F32, tag="tmin", bufs=1, name=f"tn{l}_{w}")
                    nc.vector.scalar_tensor_tensor(
                        out=tmin[:], in0=tmul[:], scalar=0.0,
                        in1=zero1[:, 0:1].broadcast_to([P, C]),
                        op0=AluOp.add, op1=AluOp.min,
                    )
                    em = wpool.tile([P, C], F32, tag="em", bufs=1, name=f"em{l}_{w}")
                    nc.scalar.activation(em[:], tmin[:], Act.Exp)
                    relu = wpool.tile([P, C], F32, tag="relu", bufs=1, name=f"rl{l}_{w}")
                    nc.vector.scalar_tensor_tensor(
                        out=relu[:], in0=tmul[:], scalar=0.0,
                        in1=zero1[:, 0:1].broadcast_to([P, C]),
                        op0=AluOp.add, op1=AluOp.max,
                    )
                    orow = wpool.tile([P, C], BF16, tag="orow", name=f"or{l}_{w}")
                    nc.vector.scalar_tensor_tensor(
                        out=orow[:], in0=em[:], scalar=-1.0, in1=relu[:],
                        op0=AluOp.add, op1=AluOp.add,
                    )
                    if l < 2:
                        nc.sync.dma_start(xrows[l][w * 128:(w + 1) * 128, :], orow[:])
                    else:
                        # fuse graph pooling: pool_ps += gsel^T @ rows
                        gw = wpool.tile([P, G], BF16, tag="gw", name=f"gw_{w}")
                        nc.vector.tensor_tensor(
                            out=gw[:], in0=iota64[:],
                            in1=gidcol[:, w: w + 1].broadcast_to([P, G]),
                            op=AluOp.is_equal,
                        )
                        nc.tensor.matmul(
                            out=pool_ps[:], lhsT=gw[:], rhs=orow[:],
                            start=(w == 0), stop=(w == NW - 1),
                        )

            def load_xT(l):
                """X^T for layer l in {1,2} via HWDGE dma-transpose of rows."""
                for b in range(4):
                    nc.sync.dma_start_transpose(
                        xT[:, b, :], xrows[l - 1][:, b * 128:(b + 1) * 128]
                    )

            def pool_fc(pool_ps):
                psums = wpool.tile([G, HID], F32, tag="psums", name="psums")
                nc.vector.tensor_copy(psums[:], pool_ps[:])
                nc.sync.dma_start(ar_in[:], psums[:])
                nc.gpsimd.collective_compute(
                    "AllReduce", AluOp.add, replica_groups=RG,
                    ins=[ar_in[:]], outs=[ar_out[:]],
                )
                sums = wpool.tile([G, HID], F32, tag="sums", name="sums")
                nc.sync.dma_start(sums[:], ar_out[:])
                pooled = wpool.tile([G, HID], F32, tag="pooled", name="pooled")
                nc.vector.tensor_scalar(
                    out=pooled[:], in0=sums[:], scalar1=invcnt[:, 0:1],
                    scalar2=None, op0=AluOp.mult,
                )
                ptp = psA.tile([HID, G], F32, tag="c0", name="poolT")
                nc.tensor.matmul(
                    out=ptp[:], lhsT=pooled[:], rhs=idf32[:G, :G],
                    start=True, stop=True, is_transpose=True,
                )
                poolT = wpool.tile([HID, G], F32, tag="poolT", name="poolTs")
                nc.vector.tensor_copy(poolT[:], ptp[:])
                pfc = psA.tile([G, OUT_CH], F32, tag="b0", name="fcps")
                nc.tensor.matmul(
                    out=pfc[:], lhsT=poolT[:], rhs=fcw[:], start=True, stop=True
                )
                logits = wpool.tile([G, OUT_CH], F32, tag="logits", name="logits")
                nc.vector.tensor_copy(logits[:], pfc[:])
                nc.sync.dma_start(out_d[:], logits[:])

            dense_phase(0)
            agg_phase(0)
            load_xT(1)
            dense_phase(1)
            agg_phase(1)
            load_xT(2)
            dense_phase(2)
            pool_ps = psA.tile([G, HID], F32, tag="d0", name="poolps")
            agg_phase(2, pool_ps)
            pool_fc(pool_ps)

            if os.environ.get("DUMP_H"):
                li = int(os.environ["DUMP_H"])
                cw = ROW1 if li < 2 else ROW3
                hstg = wpool.tile([P, cw], BF16, tag="hdmp", bufs=2, name="hdmp")
                for b in range(NP // P):
                    lo, hi = b * P, (b + 1) * P
                    nc.sync.dma_start(hstg[:], hag_out[li][lo:hi, :])
                    nc.sync.dma_start(hdump_d[lo:hi, 0:cw], hstg[:])
            if os.environ.get("DUMP_X"):
                xi = int(os.environ["DUMP_X"])  # 1 or 2: xrows after agg xi-1
                xstg = wpool.tile([P, 512], BF16, tag="xdmp", bufs=2, name="xdmp")
                for b in range(SHP // P):
                    lo, hi = b * P, (b + 1) * P
                    nc.sync.dma_start(xstg[:], xrows[xi - 1][lo:hi, :])
                    nc.sync.dma_start(xdump_d[lo:hi, :], xstg[:])

    nc.compile()
    return nc


_prog_cache = {}


def _interleave_perm():
    # perm[j] = flat channel index stored at interleaved col j
    j = np.arange(512)
    c, h = j // 4, j % 4
    return h * 128 + c


def kernel(x, edge_index, batch, W1, a_src1, a_dst1, b1,
           W2, a_src2, a_dst2, b2, W3, a_src3, a_dst3, b3, fc_w, fc_b,
           _want_results=False, _trace=False):
    x = np.asarray(x)
    edge_index = np.asarray(edge_index)
    batch = np.asarray(batch)
    for b in (b1, b2, b3, fc_b):
        assert not np.any(np.asarray(b)), "nonzero biases not supported"

    K, KW, per_core, invcnt = preprocess(edge_index, batch)
    ck = (K, tuple(KW))
    if ck not in _prog_cache:
        _prog_cache[ck] = build_program(K, KW)
    nc = _prog_cache[ck]

    iota64 = np.ascontiguousarray(
        np.broadcast_to(np.arange(G, dtype=np.float32), (P, G)).astype(BF)
    )
    idbf = np.eye(P, dtype=np.float32).astype(BF)
    idf32 = np.eye(P, dtype=np.float32)
    perm = _interleave_perm()

    def wmat(W, cinb, cout, perm_in=None):
        Wf = np.asarray(W, np.float32)
        if perm_in is not None:
            Wf = Wf[perm_in]
        return np.ascontiguousarray(
            Wf.reshape(cinb, 128, cout).transpose(1, 0, 2)
        ).astype(BF)

    w1m = wmat(W1, 2, 512)
    w2m = wmat(W2, 4, 512, perm)
    w3m = wmat(W3, 4, 128, perm)

    def avec(asrc, adst):
        nh = asrc.shape[0]
        out = np.empty((128, 2 * nh), np.float32)
        out[:, 0::2] = np.asarray(asrc, np.float32).T
        out[:, 1::2] = np.asarray(adst, np.float32).T
        return np.ascontiguousarray(out).astype(BF)

    a1m = avec(a_src1, a_dst1)
    a2m = avec(a_src2, a_dst2)
    a3m = avec(a_src3, a_dst3)
    fcw = np.ascontiguousarray(np.asarray(fc_w, np.float32))

    xf = np.asarray(x, np.float32)
    in_maps = []
    for c in range(NCORES):
        xs = np.zeros((IN_CH, SHP), np.float32)
        xs[:, :SH] = xf[c * SH: (c + 1) * SH].T
        pc = per_core[c]
        in_maps.append(
            dict(
                xT0=np.ascontiguousarray(
                    xs.reshape(2, 128, SHP).transpose(1, 0, 2)
                ).astype(BF),
                w1=w1m, w2=w2m, w3=w3m, a1=a1m, a2=a2m, a3=a3m,
                srcidx=pc["srcidx"], Omat=pc["O"], OTmat=pc["OT"],
                gidcol=pc["gidcol"],
                iota64=iota64, idbf=idbf, idf32=idf32, invcnt=invcnt, fcw=fcw,
            )
        )
    res = run_bass_kernel_spmd(
        nc, in_maps, list(range(NCORES)), trace=_trace
    )
    out = res.results[0]["logits"].astype(np.float32)
    if _want_results:
        return out, res
    return out
